# revision 1
# baseline (speedup 1.0000x reference)
"""AxialAttention (axis=height) Trainium2 Bass kernel.

Problem: x [B=2,T=4,C=256,H=128,W=128] f32. Lines run along H; N = B*T*W
independent sequences of length L=H=128 with C=256 channels, 8 heads x 32.
Sharding: one (b,t) pair per core (8 cores == B*T), i.e. data-parallel over
the leading N axis in groups of W=128 lines.

Per-core algorithm (all matmuls bf16 with fp32 PSUM accumulation):
  - x[b,t] is [C,H,W]; viewed as [C, H*W] it already IS X^T (feature-major)
    for the 16384 rows r=(h,w).
  - Stage A: q^T,k^T = (Wqk^T @ X), stored feature-major [f, (w,h)] (w-major
    so a line's 128 positions are contiguous); per line V = X_line @ Wv in
    standard [h, f] orientation, with a ones-column appended per head.
  - Attention per (line, head): scores^T[m,l] = (k^T_hd)^T @ q^T_hd  (K=32),
    probs = exp(scores^T) * exp(rel_bias^T)  (bias applied multiplicatively
    after exp: exp(s+b) = exp(s)*exp(b); values are O(1), no max-subtraction
    needed in fp32/bf16 range),
    Y[l, dh]+denominator = probs^T @ [V_hd | 1]  (lhsT=probs: full 128x128
    weight load, N=33 streaming -> near-peak PE utilization),
    normalize Y by 1/denominator (per-head broadcast multiply).
  - Y [l, c'] is PE-transposed to Y^T [c', l] and accumulated into a w-major
    block buffer; output projection OUT^T = Wout^T @ Y^T lands directly in
    the DRAM layout out[b,t] = [C,H,W].
  - bqkv biases: bq/bk added on the qk^T PSUM->SBUF copy (per-partition
    bias), softmax scale folded into the k copy; bv folded into
    bout' = bout + bv @ Wout on the host (probs sum to 1).
"""

import numpy as np
import ml_dtypes

import concourse.bacc as bacc
import concourse.bass as bass
import concourse.mybir as mybir
from concourse import tile
from concourse.bass import broadcast_tensor_aps
from concourse.bass_utils import run_bass_kernel_spmd

BF16 = ml_dtypes.bfloat16

B, T, C, H, W = 2, 4, 256, 128, 128
HEADS, DH = 8, 32
SCALE = DH ** (-0.5)
F = 3 * C  # 768
WBLK = 16
NBLK = W // WBLK  # 8
RBLK = H * WBLK  # 2048 block columns, (w, h) ordered
DT_B = mybir.dt.bfloat16
DT_F = mybir.dt.float32
AF = mybir.ActivationFunctionType


def build_program():
    nc = bacc.Bacc("TRN2")

    x_bt = nc.dram_tensor("x_bt", [C, H, W], DT_B, kind="ExternalInput")
    # packed constants: [w1 768 | w2 768 | wo1 256 | wo2 256 | expbt 1024 | id 128]
    cb16 = nc.dram_tensor("cb16", [128, 3200], DT_B, kind="ExternalInput")
    # packed f32 biases: [bqk 4 | bout2 2]
    cf32 = nc.dram_tensor("cf32", [128, 6], DT_F, kind="ExternalInput")
    out_bt = nc.dram_tensor("out_bt", [C, H, W], DT_F, kind="ExternalOutput")

    with tile.TileContext(nc) as tc:
        with (
            tc.tile_pool(name="const", bufs=1) as cpool,
            tc.tile_pool(name="xt", bufs=1) as xt_pool,
            tc.tile_pool(name="qk", bufs=12) as qk_pool,
            tc.tile_pool(name="vp", bufs=1) as v_pool,
            tc.tile_pool(name="at", bufs=10) as at_pool,
            tc.tile_pool(name="yn", bufs=4) as yn_pool,
            tc.tile_pool(name="yt", bufs=2) as yt_pool,
            tc.tile_pool(name="inv", bufs=4) as inv_pool,
            tc.tile_pool(name="outp", bufs=2) as out_pool,
            tc.tile_pool(name="psbig", bufs=2, space="PSUM") as ps_big,
            tc.tile_pool(name="pss", bufs=2, space="PSUM") as ps_s,
            tc.tile_pool(name="psy", bufs=3, space="PSUM") as ps_y,
            tc.tile_pool(name="psv", bufs=1, space="PSUM") as ps_v,
        ):
            # ---- constants (2 packed DMAs) ----
            cb = cpool.tile([128, 3200], DT_B, tag="cb16")
            nc.sync.dma_start(out=cb[:], in_=cb16[:])
            cf = cpool.tile([128, 6], DT_F, tag="cf32")
            nc.sync.dma_start(out=cf[:], in_=cf32[:])
            w1 = cb[:, 0:768]
            w2 = cb[:, 768:1536]
            wo1 = cb[:, 1536:1792]
            wo2 = cb[:, 1792:2048]
            eb_sb = cb[:, 2048:3072]
            id_sb = cb[:, 3072:3200]
            bqk_sb = cf[:, 0:4]
            bout_sb = cf[:, 4:6]

            wq = (w1, w2)

            # fence the (tiny) const loads so they never contribute sync
            # waits downstream; the big x load overlaps with compute
            tc.strict_bb_all_engine_barrier()

            # ---- resident X^T (host-cast bf16), h-quarters on two HWDGE
            # engines so stage A can start early ----
            xt_all = xt_pool.tile([128, 2 * H * W], DT_B, tag="xt")
            xt_v = xt_all[:].rearrange("p (a f) -> p a f", a=2)
            for he in range(8):
                eng = nc.sync if he % 2 == 0 else nc.scalar
                eng.dma_start(
                    out=xt_v[:, :, he * 2048 : (he + 1) * 2048],
                    in_=x_bt[:, he * 16 : (he + 1) * 16, :].rearrange(
                        "(a c) h w -> c a (h w)", a=2
                    ),
                )
            # persistent per-line V tiles; ones columns written once
            v_tiles = []
            for j in range(WBLK):
                vt = v_pool.tile([128, HEADS * 33], DT_B, tag=f"vt{j}")
                nc.vector.memset(
                    vt[:].rearrange("p (a b) -> p a b", a=HEADS, b=33)[:, :, 32],
                    1.0,
                )
                v_tiles.append(vt)

            xt0 = xt_all[:, : H * W]
            xt1 = xt_all[:, H * W :]
            # (w, h)-major view of X^T columns: [c, w, h]
            xv0 = xt0.rearrange("p (h w) -> p w h", w=W)
            xv1 = xt1.rearrange("p (h w) -> p w h", w=W)
            xvs = (xv0, xv1)

            for wb in range(NBLK):
                w0 = wb * WBLK
                # ---- stage A: q^T / k^T for this w-block, [f, (w_local, h)].
                # chunks are (8 w x 64 h) so each depends on one x h-half DMA
                qk_tiles = []
                for ft in range(4):
                    qt = qk_pool.tile([128, RBLK], DT_B, tag="qkt")
                    qk_tiles.append(qt)
                    qtv = qt[:].rearrange("p (w h) -> p w h", w=WBLK)
                    for hq in range(4):
                        ps = ps_big.tile([128, 512], DT_F, tag="big")
                        for ct in range(2):
                            nc.tensor.matmul(
                                ps[:],
                                lhsT=wq[ct][:, ft * 128 : (ft + 1) * 128],
                                rhs=xvs[ct][
                                    :,
                                    w0 : w0 + WBLK,
                                    hq * 32 : (hq + 1) * 32,
                                ],
                                start=(ct == 0),
                                stop=(ct == 1),
                            )
                        nc.scalar.activation(
                            qtv[:, :, hq * 32 : (hq + 1) * 32],
                            ps[:].rearrange("p (w h) -> p w h", w=WBLK),
                            AF.Identity,
                            bias=bqk_sb[:, ft : ft + 1],
                            scale=(SCALE if ft >= 2 else 1.0),
                        )

                # ---- V per line (standard orientation [h, f_v]), ones col per head ----
                for j in range(WBLK):
                    w = w0 + j
                    ps = ps_v.tile([128, 256], DT_F, tag="vps")
                    for ct in range(2):
                        nc.tensor.matmul(
                            ps[:],
                            lhsT=xvs[ct][:, w, :],
                            rhs=wq[ct][:, 512:768],
                            start=(ct == 0),
                            stop=(ct == 1),
                        )
                    vt3 = v_tiles[j][:].rearrange("p (a b) -> p a b", a=HEADS, b=33)
                    nc.vector.tensor_copy(
                        vt3[:, :, 0:32],
                        ps[:].rearrange("p (a b) -> p a b", a=HEADS, b=32),
                    )

                # ---- attention per line ----
                yt = yt_pool.tile([128, 2 * RBLK], DT_B, tag="yt")
                def qk_pair_duo(j, p):
                    # heads (p, p+4) x lines (j, j+1): all four matmuls use
                    # PE row group r0 = p*32, so they serialize through the
                    # same column ports and may share one PSUM bank safely
                    r0 = p * 32
                    ps = ps_s.tile([128, 512], DT_F, tag="sc")
                    for lh in range(2):
                        jc = slice((j + lh) * 128, (j + lh + 1) * 128)
                        for half in range(2):
                            nc.tensor.matmul(
                                ps[:, lh * 256 + half * 128 : lh * 256 + (half + 1) * 128],
                                lhsT=qk_tiles[2 + half][r0 : r0 + 32, jc],
                                rhs=qk_tiles[half][r0 : r0 + 32, jc],
                                start=True,
                                stop=True,
                                tile_position=(r0, 0),
                            )
                    exps = at_pool.tile([128, 512], DT_B, tag="exps")
                    nc.scalar.activation(exps[:], ps[:], AF.Exp)
                    attnw = at_pool.tile([128, 512], DT_B, tag="attnw")
                    eb3 = eb_sb[:, p * 256 : (p + 1) * 256].rearrange(
                        "p (a c) -> p a c", a=1
                    )
                    aw3 = attnw[:].rearrange("p (a c) -> p a c", a=2)
                    e3 = exps[:].rearrange("p (a c) -> p a c", a=2)
                    i0, i1 = broadcast_tensor_aps(e3, eb3)
                    nc.vector.tensor_tensor(aw3, i0, i1, mybir.AluOpType.mult)
                    return attnw

                def av_pair(j, y_ps, attnw, lh, p):
                    # rhs=[V_hd | 1] adds the softmax denominator as
                    # output column 32 of each head's 33-col group
                    for half in range(2):
                        hd = p + 4 * half
                        nc.tensor.matmul(
                            y_ps[:, hd * 33 : hd * 33 + 33],
                            lhsT=attnw[:, lh * 256 + half * 128 : lh * 256 + (half + 1) * 128],
                            rhs=v_tiles[j][:, hd * 33 : hd * 33 + 33],
                            start=True,
                            stop=True,
                        )

                def line_tail(j, y_ps):
                    # normalize by the denominators, transpose to Y^T, and
                    # store into the block buffer
                    jc = slice(j * 128, (j + 1) * 128)
                    y3 = y_ps[:].rearrange("p (a b) -> p a b", a=HEADS, b=33)
                    invd = inv_pool.tile([128, HEADS], DT_F, tag="invd")
                    nc.vector.reciprocal(invd[:], y3[:, :, 32])
                    yn = yn_pool.tile([128, C], DT_B, tag="yn")
                    i0, i1 = broadcast_tensor_aps(
                        y3[:, :, 0:32],
                        invd[:].rearrange("p (a b) -> p a b", b=1),
                    )
                    nc.vector.tensor_tensor(
                        yn[:].rearrange("p (a b) -> p a b", a=HEADS, b=32),
                        i0,
                        i1,
                        mybir.AluOpType.mult,
                    )
                    tr = ps_y.tile([128, 256], DT_B, tag="ypsy")
                    nc.tensor.transpose(tr[:, 0:128], yn[:, 0:128], id_sb[:])
                    nc.tensor.transpose(tr[:, 128:256], yn[:, 128:256], id_sb[:])
                    nc.vector.tensor_copy(
                        yt[:]
                        .rearrange("p (a l) -> p a l", a=2)[:, :, jc],
                        tr[:].rearrange("p (a l) -> p a l", a=2),
                    )

                for j in range(0, WBLK, 2):
                    y_ps0 = ps_y.tile([128, HEADS * 33], DT_F, tag="ypsy")
                    y_ps1 = ps_y.tile([128, HEADS * 33], DT_F, tag="ypsy")
                    aws = [qk_pair_duo(j, p) for p in range(4)]
                    for p in range(4):
                        av_pair(j, y_ps0, aws[p], 0, p)
                        av_pair(j + 1, y_ps1, aws[p], 1, p)
                    line_tail(j, y_ps0)
                    line_tail(j + 1, y_ps1)

                # ---- output projection for this block ----
                for ct in range(2):
                    # ot holds the chunk in DRAM order: cols (h, w_local)
                    ot = out_pool.tile([128, RBLK], DT_F, tag="ot")
                    otv = ot[:].rearrange("p (h w) -> p h w", w=WBLK)
                    for ch in range(RBLK // 512):
                        ps = ps_y.tile([128, 512], DT_F, tag="ypsy")
                        nc.tensor.matmul(
                            ps[:],
                            lhsT=wo1[:, ct * 128 : (ct + 1) * 128],
                            rhs=yt[:, ch * 512 : (ch + 1) * 512],
                            start=True,
                            stop=False,
                        )
                        nc.tensor.matmul(
                            ps[:],
                            lhsT=wo2[:, ct * 128 : (ct + 1) * 128],
                            rhs=yt[:, RBLK + ch * 512 : RBLK + (ch + 1) * 512],
                            start=False,
                            stop=True,
                        )
                        # psum cols are (w 4, h 128); write reordered to (h, w)
                        nc.scalar.activation(
                            otv[:, :, ch * 4 : (ch + 1) * 4],
                            ps[:].rearrange("p (w h) -> p h w", w=4),
                            AF.Identity,
                            bias=bout_sb[:, ct : ct + 1],
                        )
                    nc.sync.dma_start(
                        out=out_bt[ct * 128 : (ct + 1) * 128, :, w0 : w0 + WBLK],
                        in_=ot[:],
                    )

    nc.compile()
    return nc


_NC = None


def _get_nc():
    global _NC
    if _NC is None:
        _NC = build_program()
    return _NC


def _prep_small(rel_bias, Wqkv, bqkv, Wout, bout):
    # bf16 blob: [w1 768 | w2 768 | wo1 256 | wo2 256 | expbt 1024 | id 128]
    w12 = Wqkv.reshape(2, 128, F)
    wo12 = Wout.reshape(2, 128, C)
    expbt_a = np.exp(rel_bias.transpose(0, 2, 1))  # [hd, m, l]
    # head order (0,4),(1,5),(2,6),(3,7): pair (hd, hd+4) shares a PE row
    # group, so the pair's scores can share one PSUM tile safely
    expbt_a = expbt_a[[0, 4, 1, 5, 2, 6, 3, 7]]
    eb = expbt_a.transpose(1, 0, 2).reshape(128, HEADS * 128)  # [m, (hd, l)]
    cb16 = np.concatenate(
        [w12[0], w12[1], wo12[0], wo12[1], eb, np.eye(128, dtype=np.float32)],
        axis=1,
    ).astype(BF16)
    bqk_a = np.stack(
        [
            bqkv[0:128],
            bqkv[128:256],
            SCALE * bqkv[256:384],
            SCALE * bqkv[384:512],
        ],
        axis=1,
    )
    bout2_a = (bout + bqkv[512:] @ Wout).reshape(2, 128).T
    cf32 = np.concatenate([bqk_a, bout2_a], axis=1).astype(np.float32)
    return {"cb16": np.ascontiguousarray(cb16), "cf32": np.ascontiguousarray(cf32)}


def _run(x, rel_bias, Wqkv, bqkv, Wout, bout, **spmd_kwargs):
    x = np.asarray(x, dtype=np.float32)
    small = _prep_small(
        np.asarray(rel_bias, np.float32),
        np.asarray(Wqkv, np.float32),
        np.asarray(bqkv, np.float32),
        np.asarray(Wout, np.float32),
        np.asarray(bout, np.float32),
    )
    nc = _get_nc()
    core_ids = list(range(8))
    in_maps = []
    for i in core_ids:
        b, t = divmod(i, T)
        m = dict(small)
        m["x_bt"] = np.ascontiguousarray(x[b, t]).astype(BF16)
        in_maps.append(m)
    res = run_bass_kernel_spmd(nc, in_maps, core_ids, **spmd_kwargs)
    out = np.empty((B, T, C, H, W), np.float32)
    for i in core_ids:
        b, t = divmod(i, T)
        out[b, t] = res.results[i]["out_bt"]
    return out, res


def kernel(x, rel_bias, Wqkv, bqkv, Wout, bout):
    out, _ = _run(x, rel_bias, Wqkv, bqkv, Wout, bout)
    return out

